# revision 1
# baseline (speedup 1.0000x reference)
"""EntNet forward on 8 Trainium2 NeuronCores.

Strategy (data-parallel over batch B, per sharding hint):
- Each core owns BS = B/8 = 16 batch rows -> JB = J*BS = 320 independent
  (j,b) scan columns, kept E-major: tiles are [E=128 partitions, 320 free].
- The T=512-step recurrence runs fully on-device per core; embedding rows
  are gathered + transposed on host into textembT [E, T*BS] and kept
  resident in SBUF.
- All matmuls use fp32r (full-rate on the PE at N=320) accumulating fp32
  in PSUM. Reductions over E (gate logits, squared norms) are PE matmuls
  against an all-ones / replicated-v stationary matrix, which also
  broadcasts the row result across all 128 partitions for free.
- sigmoid/tanh/relu/square run on ScalarE (single table set). The
  per-column rsqrt for h-normalization uses an exponent-halving seed
  (int32->fp32 convert-copy, affine with the 0x5f3759df magic, fp32->int32
  convert-on-write) plus NR_PASSES Newton-Raphson passes in stock ops.
- The tiny output head (attention over J, BN over full B, 2 linears) is
  computed on host from the final h state.
"""
import sys

sys.path.insert(0, "/opt/trn_rl_repo")

import numpy as np

import concourse.bass as bass
import concourse.mybir as mybir
from concourse import tile
from concourse.bass_utils import run_bass_kernel_spmd
from concourse.vector_clock import ScopedClock

# ---------------------------------------------------------------- tile patch
# This container's walrus build rejects CTRL-class instructions carrying
# more than a couple of sync waits; Tile's kernel-tail drain aggregates one
# wait per proc. Split them across a chain of NOPs (same semantics).


def _patched_drain_and_barrier(self, tick_clock, wait_clock):
    MAXW = 1
    probe = self.nc.sync.nop(nofuse=True, hint="drain_wait_split")
    wait_clock.add_sem_waits(
        probe.ins, ScopedClock({None: tick_clock.global_clock})
    )
    si = probe.ins.sync_info
    waits = list(si.on_wait) if si and si.on_wait else []
    if len(waits) > MAXW:
        probe.ins.sync_info = mybir.SyncInfo(
            on_wait=waits[:MAXW], on_update=si.on_update if si else []
        )
        rest = waits[MAXW:]
        for i in range(0, len(rest), MAXW):
            n2 = self.nc.sync.nop(nofuse=True, hint="drain_wait_split")
            prev = n2.ins.sync_info
            n2.ins.sync_info = mybir.SyncInfo(
                on_wait=(list(prev.on_wait) if prev and prev.on_wait else [])
                + rest[i : i + MAXW],
                on_update=prev.on_update if prev else [],
            )
    self.nc.sync.drain()
    self.nc.all_engine_barrier()
    assert self.sems is not None
    popped = self.nc._tile_sem_poison_stack.pop()
    assert popped is self._sem_poison
    self.nc.clear_and_free_semaphores(list(self.sems.allocated().values()))
    self.nc.all_engine_barrier()


tile.TileContext._drain_and_barrier = _patched_drain_and_barrier

_MAXW = 1
_split_ctr = [0]


def _split_sync_waits(nc):
    """Post-pass: this walrus build rejects instructions carrying more than
    ~2 sync waits. Move excess waits onto fresh NoOps inserted just before
    the offending instruction on the same engine (in-order execution makes
    this equivalent)."""
    for fn in nc.m.functions:
        for bb in fn.blocks:
            newlist = []
            for inst in bb.instructions:
                si = inst.sync_info
                w = list(si.on_wait) if si and si.on_wait else []
                if len(w) > _MAXW:
                    for i in range(0, len(w) - _MAXW, _MAXW):
                        _split_ctr[0] += 1
                        newlist.append(mybir.InstNoOp(
                            name=f"I-waitsplit-{_split_ctr[0]}",
                            engine=inst.engine,
                            bass_nofuse=True,
                            sync_info=mybir.SyncInfo(
                                on_wait=w[i:i + _MAXW], on_update=[]),
                        ))
                    inst.sync_info = mybir.SyncInfo(
                        on_wait=w[len(w) - _MAXW:],
                        on_update=si.on_update if si else [])
                newlist.append(inst)
            bb.instructions[:] = newlist

# ------------------------------------------------------------------- shapes
T, B, V, E, J, L = 512, 128, 50000, 128, 20, 3
NC_ = 8
BS = B // NC_          # 16
JB = J * BS            # 320
UNROLL = 4             # steps per For_i body (ping-pong state)
USE_GP_BLEND = False   # run d-blend + abm on GpSimd
ACT_WJT = False        # wJT broadcast copy on ScalarE (else VectorE)
STAGGERED = False
WP_BUFS = 2
PMCAND_BUFS = 1        # measured: double-buffering the cand PSUM bank hurts
NR_PASSES = 2          # Newton passes for rsqrt (2 -> ~4e-6 norm err)
MAGIC = np.float32(1597463007.0)  # 0x5f3759df as float

F32 = mybir.dt.float32
F32R = mybir.dt.float32r
I32 = mybir.dt.int32


def _r(ap):
    return ap


def _build_nc(mm_fp32r=True, n_steps=T, split_waits=True, repeat=1):
    nc = bass.Bass("TRN2", num_devices=NC_, debug=False,
                   enable_asserts=False, target_bir_lowering=False)

    din = {}
    MDT = F32R if mm_fp32r else F32
    def inp(name, shape, dt=F32):
        din[name] = nc.dram_tensor(name, list(shape), dt, kind="ExternalInput")
        return din[name]

    inp("textembT", (E, T * BS))
    inp("h0T", (E, JB), MDT)
    inp("d0T", (E, JB), MDT)
    inp("U", (E, E), MDT)
    inp("W", (E, E), MDT)
    inp("ident", (E, E), MDT)
    inp("WihT", (E, 3 * E), MDT)
    inp("WhhT", (E, 3 * E), MDT)
    inp("kVJT", (E, JB), MDT)
    inp("keysJT", (E, JB))
    inp("ones", (E, E), MDT)
    inp("vmat", (E, E), MDT)
    # bias columns: 0 U_bias, 1 b_r, 2 b_z, 3 bih_n, 4 bhh_n, 5 magic, 6 v
    inp("bcols", (E, 8))
    out_h = nc.dram_tensor("hT_fin", [E, JB], F32, kind="ExternalOutput")

    mr = _r if mm_fp32r else (lambda ap: ap)

    with tile.TileContext(nc) as tc:
        with (
            tc.tile_pool(name="const", bufs=1) as cp,
            tc.tile_pool(name="state", bufs=1) as sp,
            tc.tile_pool(name="work", bufs=WP_BUFS) as wp,
            tc.tile_pool(name="psum", bufs=1, space="PSUM") as pp,
        ):
            # ---- load constants
            t_emb = cp.tile([E, T * BS], F32)
            nc.sync.dma_start(t_emb[:, :], din["textembT"].ap())
            t_U = cp.tile([E, E], MDT)
            nc.sync.dma_start(t_U[:, :], din["U"].ap())
            t_W = cp.tile([E, E], MDT)
            nc.sync.dma_start(t_W[:, :], din["W"].ap())
            t_I = cp.tile([E, E], MDT)
            nc.sync.dma_start(t_I[:, :], din["ident"].ap())
            t_Wih = cp.tile([E, 3 * E], MDT)
            nc.sync.dma_start(t_Wih[:, :], din["WihT"].ap())
            t_Whh = cp.tile([E, 3 * E], MDT)
            nc.sync.dma_start(t_Whh[:, :], din["WhhT"].ap())
            t_kVJT = cp.tile([E, JB], MDT)
            nc.sync.dma_start(t_kVJT[:, :], din["kVJT"].ap())
            t_keysJT = cp.tile([E, JB], F32)
            nc.sync.dma_start(t_keysJT[:, :], din["keysJT"].ap())
            t_bc = cp.tile([E, 8], F32)
            nc.sync.dma_start(t_bc[:, :], din["bcols"].ap())

            ub = t_bc[:, 0:1]
            b_r = t_bc[:, 1:2]
            b_z = t_bc[:, 2:3]
            bih_n = t_bc[:, 3:4]
            bhh_n = t_bc[:, 4:5]
            magic = t_bc[:, 5:6]

            # ones [E,128] and vmat [E,128] (v replicated) from DRAM
            t_ones = cp.tile([E, E], MDT)
            nc.sync.dma_start(t_ones[:, :], din["ones"].ap())
            t_vmat = cp.tile([E, E], MDT)
            nc.sync.dma_start(t_vmat[:, :], din["vmat"].ap())

            # ---- state (ping-pong)
            t_h = [sp.tile([E, JB], MDT, name=f"h{i}", tag=f"h{i}") for i in range(2)]
            t_d = [sp.tile([E, JB], MDT, name=f"d{i}", tag=f"d{i}") for i in range(2)]
            nc.sync.dma_start(t_h[0][:, :], din["h0T"].ap())
            nc.sync.dma_start(t_d[0][:, :], din["d0T"].ap())

            def step(tcol, h_in, d_in, h_out, d_out):
                """one scan step; tcol = ScalarValue column offset (t*BS)"""
                wsl = t_emb[:, bass.ds(tcol, BS)]

                # broadcast w over j -> wJT [E, JB]
                wJT = wp.tile([E, JB], MDT, tag="wJT")
                (nc.scalar.copy if ACT_WJT else nc.vector.tensor_copy)(
                    wJT[:, :].rearrange("e (j b) -> e j b", j=J),
                    wsl.unsqueeze(1).broadcast_to([E, J, BS]),
                )

                # candidate pre-activation
                pm_cand = pp.tile([E, JB], F32, tag="pm_cand", bufs=PMCAND_BUFS)
                nc.tensor.matmul(pm_cand[:, :], mr(t_U[:, :]), mr(h_in[:, :]),
                                 start=True, stop=False)
                nc.tensor.matmul(pm_cand[:, :], mr(t_W[:, :]), mr(wJT[:, :]),
                                 start=False, stop=False)
                nc.tensor.matmul(pm_cand[:, :], mr(t_I[:, :]), mr(t_kVJT[:, :]),
                                 start=False, stop=True)
                candi = wp.tile([E, JB], MDT, tag="candi")
                nc.scalar.activation(candi[:, :], pm_cand[:, :],
                                     mybir.ActivationFunctionType.Relu,
                                     bias=ub)

                # GRU gate pre-activations
                pm_r = pp.tile([E, JB], F32, tag="pm_r")
                nc.tensor.matmul(pm_r[:, :], mr(t_Whh[:, 0:E]), mr(d_in[:, :]),
                                 start=True, stop=False)
                nc.tensor.matmul(pm_r[:, :], mr(t_Wih[:, 0:E]), mr(candi[:, :]),
                                 start=False, stop=True)
                pm_z = pp.tile([E, JB], F32, tag="pm_z")
                nc.tensor.matmul(pm_z[:, :], mr(t_Whh[:, E:2 * E]), mr(d_in[:, :]),
                                 start=True, stop=False)
                nc.tensor.matmul(pm_z[:, :], mr(t_Wih[:, E:2 * E]), mr(candi[:, :]),
                                 start=False, stop=True)
                pm_hn = pp.tile([E, JB], F32, tag="pm_hn")
                nc.tensor.matmul(pm_hn[:, :], mr(t_Whh[:, 2 * E:]), mr(d_in[:, :]),
                                 start=True, stop=True)
                pm_inn = pp.tile([E, JB], F32, tag="pm_inn")
                nc.tensor.matmul(pm_inn[:, :], mr(t_Wih[:, 2 * E:]), mr(candi[:, :]),
                                 start=True, stop=True)

                rT = wp.tile([E, JB], F32, tag="rT")
                nc.scalar.activation(rT[:, :], pm_r[:, :],
                                     mybir.ActivationFunctionType.Sigmoid,
                                     bias=b_r)
                zT = wp.tile([E, JB], F32, tag="zT")
                nc.scalar.activation(zT[:, :], pm_z[:, :],
                                     mybir.ActivationFunctionType.Sigmoid,
                                     bias=b_z)

                # n = tanh((inn + bih_n) + r*(hn + bhh_n))
                rhn = wp.tile([E, JB], F32, tag="rhn")
                nc.vector.scalar_tensor_tensor(
                    rhn[:, :], pm_hn[:, :], bhh_n, rT[:, :],
                    op0=mybir.AluOpType.add, op1=mybir.AluOpType.mult)
                tadd = wp.tile([E, JB], F32, tag="tadd")
                nc.vector.scalar_tensor_tensor(
                    tadd[:, :], pm_inn[:, :], bih_n, rhn[:, :],
                    op0=mybir.AluOpType.add, op1=mybir.AluOpType.add)
                nT = wp.tile([E, JB], F32, tag="nT")
                nc.scalar.activation(nT[:, :], tadd[:, :],
                                     mybir.ActivationFunctionType.Tanh)

                # new_d = n + z*(d - n)
                t1 = wp.tile([E, JB], F32, tag="t1")
                (nc.gpsimd if USE_GP_BLEND else nc.vector).tensor_tensor(t1[:, :], d_in[:, :], nT[:, :],
                                        op=mybir.AluOpType.subtract)
                t2 = wp.tile([E, JB], F32, tag="t2")
                (nc.gpsimd if USE_GP_BLEND else nc.vector).tensor_tensor(t2[:, :], zT[:, :], t1[:, :],
                                        op=mybir.AluOpType.mult)
                (nc.gpsimd if USE_GP_BLEND else nc.vector).tensor_tensor(d_out[:, :], nT[:, :], t2[:, :],
                                        op=mybir.AluOpType.add)

                # gate logit = sum_e (h+keysJT)*wJT + v . new_d  (bcast over E)
                abm = wp.tile([E, JB], F32, tag="abm")
                (nc.gpsimd if USE_GP_BLEND else nc.vector).tensor_tensor(abm[:, :], h_in[:, :], t_keysJT[:, :],
                                        op=mybir.AluOpType.add)
                abm2 = wp.tile([E, JB], MDT, tag="abm2")
                nc.vector.tensor_tensor(
                    abm2[:, :].rearrange("e (j b) -> e j b", j=J),
                    abm[:, :].rearrange("e (j b) -> e j b", j=J),
                    wsl.unsqueeze(1).broadcast_to([E, J, BS]),
                    op=mybir.AluOpType.mult)
                pm_gate = pp.tile([E, JB], F32, tag="pm_gate")
                nc.tensor.matmul(pm_gate[:, :], mr(t_ones[:, :]), mr(abm2[:, :]),
                                 start=True, stop=False)
                nc.tensor.matmul(pm_gate[:, :], mr(t_vmat[:, :]), mr(d_out[:, :]),
                                 start=False, stop=True)
                gateE = wp.tile([E, JB], F32, tag="gateE")
                nc.scalar.activation(gateE[:, :], pm_gate[:, :],
                                     mybir.ActivationFunctionType.Sigmoid)

                # h1 = h + gate*candi
                gc = wp.tile([E, JB], F32, tag="gc")
                nc.vector.tensor_tensor(gc[:, :], gateE[:, :], candi[:, :],
                                        op=mybir.AluOpType.mult)
                h1 = wp.tile([E, JB], F32, tag="h1")
                nc.vector.tensor_tensor(h1[:, :], h_in[:, :], gc[:, :],
                                        op=mybir.AluOpType.add)

                # column norm: ss = sum_e h1^2 (bcast), rn = rsqrt(ss)
                sq = wp.tile([E, JB], MDT, tag="sq")
                nc.scalar.activation(sq[:, :], h1[:, :],
                                     mybir.ActivationFunctionType.Square)
                pm_ss = pp.tile([E, JB], F32, tag="pm_ss")
                nc.tensor.matmul(pm_ss[:, :], mr(t_ones[:, :]), mr(sq[:, :]),
                                 start=True, stop=True)
                seedf = wp.tile([E, JB], F32, tag="seedf")
                nc.scalar.activation(seedf[:, :], pm_ss[:, :].bitcast(I32),
                                     mybir.ActivationFunctionType.Copy)
                y0i = wp.tile([E, JB], I32, tag="y0i")
                nc.scalar.activation(y0i[:, :], seedf[:, :],
                                     mybir.ActivationFunctionType.Identity,
                                     bias=magic, scale=-0.5)
                # Newton passes with stock ops: y' = y*(1.5 - 0.5*x*y^2)
                y = y0i[:, :].bitcast(F32)
                for k in range(NR_PASSES):
                    ysq = wp.tile([E, JB], F32, tag=f"ysq{k}")
                    nc.scalar.activation(ysq[:, :], y,
                                         mybir.ActivationFunctionType.Square)
                    hxy = wp.tile([E, JB], F32, tag=f"hxy{k}")
                    nc.vector.scalar_tensor_tensor(
                        hxy[:, :], ysq[:, :], -0.5, pm_ss[:, :],
                        op0=mybir.AluOpType.mult, op1=mybir.AluOpType.mult)
                    ynew = wp.tile([E, JB], F32, tag=f"ynew{k}")
                    nc.vector.scalar_tensor_tensor(
                        ynew[:, :], hxy[:, :], 1.5, y,
                        op0=mybir.AluOpType.add, op1=mybir.AluOpType.mult)
                    y = ynew[:, :]
                nc.vector.tensor_tensor(h_out[:, :], h1[:, :], y,
                                        op=mybir.AluOpType.mult)

            with tc.For_i(0, repeat, 1) as _rep:
                with tc.For_i(0, n_steps // UNROLL, 1,
                              staggered_reset=STAGGERED) as it:
                    base = it * (UNROLL * BS)
                    for u in range(UNROLL):
                        step(base + u * BS,
                             t_h[u % 2], t_d[u % 2],
                             t_h[(u + 1) % 2], t_d[(u + 1) % 2])

            nc.sync.dma_start(out_h.ap(), t_h[0][:, :].bitcast(F32))

    if split_waits:
        _split_sync_waits(nc)
    return nc


# ------------------------------------------------------------ host wrappers
_CACHE = {}


def _get_nc():
    if "nc" not in _CACHE:
        _CACHE["nc"] = _build_nc()
    return _CACHE["nc"]


def _prep_core_inputs(c, text, emb, shared):
    bs, be = c * BS, (c + 1) * BS
    tcore = text[:, bs:be]
    gat = emb[tcore.reshape(-1)]                    # [T*BS, E]
    textembT = np.ascontiguousarray(gat.T)          # [E, T*BS]
    m = dict(shared["consts"])
    m["textembT"] = textembT
    m["h0T"] = np.ascontiguousarray(
        shared["h0"][:, bs:be, :].transpose(2, 0, 1).reshape(E, JB))
    m["d0T"] = np.ascontiguousarray(
        shared["d0"][:, bs:be, :].transpose(2, 0, 1).reshape(E, JB))
    return m


def _make_shared(inputs):
    f32 = np.float32
    keys, Vm, v = inputs["keys"], inputs["Vm"], inputs["v"]
    bih, bhh = inputs["bih"], inputs["bhh"]
    kV = (keys @ Vm).astype(f32)                    # [J,E]
    bcols = np.zeros((E, 8), f32)
    bcols[:, 0] = inputs["U_bias"]
    bcols[:, 1] = bih[:E] + bhh[:E]
    bcols[:, 2] = bih[E:2 * E] + bhh[E:2 * E]
    bcols[:, 3] = bih[2 * E:]
    bcols[:, 4] = bhh[2 * E:]
    bcols[:, 5] = MAGIC
    bcols[:, 6] = v
    consts = {
        "U": np.ascontiguousarray(inputs["U"], dtype=f32),
        "W": np.ascontiguousarray(inputs["W"], dtype=f32),
        "ident": np.eye(E, dtype=f32),
        "WihT": np.ascontiguousarray(np.asarray(inputs["Wih"]).T, dtype=f32),
        "WhhT": np.ascontiguousarray(np.asarray(inputs["Whh"]).T, dtype=f32),
        "kVJT": np.repeat(kV.T, BS, axis=1).astype(f32),
        "keysJT": np.repeat(np.asarray(keys).T, BS, axis=1).astype(f32),
        "bcols": bcols,
        "ones": np.ones((E, E), f32),
        "vmat": np.tile(np.asarray(v, dtype=f32)[:, None], (1, E)),
    }
    return {"consts": consts, "h0": np.asarray(inputs["h0"], dtype=f32),
            "d0": np.asarray(inputs["d0"], dtype=f32)}


def kernel(text, target, aspect, emb, keys, U, Vm, W, U_bias, v,
           Wih, Whh, bih, bhh, W_att, c1_w, c1_b, bn_g, bn_b,
           c2_w, c2_b, h0, d0):
    text = np.asarray(text)
    emb = np.asarray(emb, dtype=np.float32)
    f32 = np.float32

    shared = _make_shared(dict(
        text=text, target=target, aspect=aspect, emb=emb, keys=keys, U=U,
        Vm=Vm, W=W, U_bias=U_bias, v=v, Wih=Wih, Whh=Whh, bih=bih, bhh=bhh,
        W_att=W_att, c1_w=c1_w, c1_b=c1_b, bn_g=bn_g, bn_b=bn_b, c2_w=c2_w,
        c2_b=c2_b, h0=h0, d0=d0))

    in_maps = [_prep_core_inputs(c, text, emb, shared) for c in range(NC_)]

    nc = _get_nc()
    res = run_bass_kernel_spmd(nc, in_maps, core_ids=list(range(NC_)))
    _CACHE["last_results"] = res

    h_fin = np.zeros((J, B, E), f32)
    for c in range(NC_):
        hT = res.results[c]["hT_fin"]               # [E, JB]
        h_fin[:, c * BS:(c + 1) * BS, :] = (
            hT.reshape(E, J, BS).transpose(1, 2, 0))

    # ---- output head (host)
    target_embed = emb[np.asarray(target)]
    aspect_embed = emb[np.asarray(aspect)]
    last_h = h_fin.transpose(1, 0, 2)               # [B,J,E]
    ta = np.concatenate([target_embed, aspect_embed], axis=1)
    att = ((np.asarray(keys) @ np.asarray(W_att)) @ ta.T).T.astype(f32)
    att = att - att.max(axis=1, keepdims=True)
    att = np.exp(att)
    att /= att.sum(axis=1, keepdims=True)
    u_read = np.einsum("bje,bj->be", last_h, att).astype(f32)
    hidden = u_read @ np.asarray(c1_w).T + c1_b + aspect_embed
    mu = hidden.mean(axis=0)
    var = hidden.var(axis=0)
    hidden = (hidden - mu) / np.sqrt(var + 1e-5) * bn_g + bn_b
    hidden = np.maximum(hidden, 0)
    return (hidden @ np.asarray(c2_w).T + c2_b).astype(f32)



# revision 2
# speedup vs baseline: 3.9277x; 3.9277x over previous
"""EntNet forward on 8 Trainium2 NeuronCores — v2 (restructured).

Changes vs baseline:
- All matmuls bf16 (1 cyc/row at any width); state h/d in bf16.
- 2 independent column streams (j 0..9 | j 10..19, W=160 each) emitted
  op-interleaved so each stream's serial chain hides the other's hops.
- Shorter critical chain: om_z = sigmoid(-x) complement; gate C-term via
  vm@omn (+vm@zd early) instead of materializing d' on the path; d' blend
  on GpSimd off-path; keys.w gate term precomputed on host and injected
  into PSUM via a 1-partition matmul; U_bias folded into kV; GRU biases
  folded into activation bias columns.
- rsqrt: magic-seed only (NR0) as one DVE stt (int read, float compute,
  int write); numerics validated end-to-end at 2.8e-3 rel err.
"""
import sys

sys.path.insert(0, "/opt/trn_rl_repo")

import numpy as np
import ml_dtypes

import concourse.bass as bass
import concourse.mybir as mybir
from concourse import tile
from concourse.bass_utils import run_bass_kernel_spmd
from concourse.vector_clock import ScopedClock

BF = ml_dtypes.bfloat16

# ---------------------------------------------------------------- tile patch
# This container's walrus build rejects CTRL-class instructions carrying
# more than a couple of sync waits; split them across NOP chains.


def _patched_drain_and_barrier(self, tick_clock, wait_clock):
    MAXW = 1
    probe = self.nc.sync.nop(nofuse=True, hint="drain_wait_split")
    wait_clock.add_sem_waits(
        probe.ins, ScopedClock({None: tick_clock.global_clock})
    )
    si = probe.ins.sync_info
    waits = list(si.on_wait) if si and si.on_wait else []
    if len(waits) > MAXW:
        probe.ins.sync_info = mybir.SyncInfo(
            on_wait=waits[:MAXW], on_update=si.on_update if si else []
        )
        rest = waits[MAXW:]
        for i in range(0, len(rest), MAXW):
            n2 = self.nc.sync.nop(nofuse=True, hint="drain_wait_split")
            prev = n2.ins.sync_info
            n2.ins.sync_info = mybir.SyncInfo(
                on_wait=(list(prev.on_wait) if prev and prev.on_wait else [])
                + rest[i : i + MAXW],
                on_update=prev.on_update if prev else [],
            )
    self.nc.sync.drain()
    self.nc.all_engine_barrier()
    assert self.sems is not None
    popped = self.nc._tile_sem_poison_stack.pop()
    assert popped is self._sem_poison
    self.nc.clear_and_free_semaphores(list(self.sems.allocated().values()))
    self.nc.all_engine_barrier()


tile.TileContext._drain_and_barrier = _patched_drain_and_barrier

_MAXW = 1
_split_ctr = [0]


def _split_sync_waits(nc):
    for fn in nc.m.functions:
        for bb in fn.blocks:
            newlist = []
            for inst in bb.instructions:
                si = inst.sync_info
                w = list(si.on_wait) if si and si.on_wait else []
                if len(w) > _MAXW:
                    for i in range(0, len(w) - _MAXW, _MAXW):
                        _split_ctr[0] += 1
                        newlist.append(mybir.InstNoOp(
                            name=f"I-waitsplit-{_split_ctr[0]}",
                            engine=inst.engine,
                            bass_nofuse=True,
                            sync_info=mybir.SyncInfo(
                                on_wait=w[i:i + _MAXW], on_update=[]),
                        ))
                    inst.sync_info = mybir.SyncInfo(
                        on_wait=w[len(w) - _MAXW:],
                        on_update=si.on_update if si else [])
                newlist.append(inst)
            bb.instructions[:] = newlist

# ------------------------------------------------------------------- shapes
T, B, V, E, J, L = 512, 128, 50000, 128, 20, 3
NC_ = 8
BS = B // NC_            # 16
JB = J * BS              # 320
NS = 2                   # streams
JS = J // NS             # 10 j-blocks per stream
W = JS * BS              # 160 columns per stream
UNROLL = 4
NR_PASSES = 0            # magic seed only (validated 2.8e-3 end-to-end)
MAGICF = float(np.float32(1597463007.0))

F32 = mybir.dt.float32
BF16 = mybir.dt.bfloat16
I32 = mybir.dt.int32
ALU = mybir.AluOpType
ACT = mybir.ActivationFunctionType


def _build_nc(n_steps=T, split_waits=True, repeat=1, nr_passes=NR_PASSES,
              unroll=UNROLL, staggered=False):
    nc = bass.Bass("TRN2", num_devices=NC_, debug=False,
                   enable_asserts=False, target_bir_lowering=False)

    din = {}
    def inp(name, shape, dt=BF16):
        din[name] = nc.dram_tensor(name, list(shape), dt, kind="ExternalInput")
        return din[name]

    inp("textembT", (E, T * BS))
    inp("h0T", (E, JB))
    inp("d0T", (E, JB))
    inp("U", (E, E))
    inp("W", (E, E))
    inp("ident", (E, E))
    inp("ones", (E, E))
    inp("vmat", (E, E))
    inp("WihT", (E, 3 * E))
    inp("WhhT", (E, 3 * E))
    inp("kvub", (E, JB))
    inp("b4", (unroll, (T // unroll) * JB))
    inp("oh4", (unroll, unroll * E))
    # bias columns (f32): 0 b_r, 1 b_z, 2 nb_z(=-b_z), 3 bih_n, 4 bhh_n, 5 magic
    inp("bcols", (E, 8), F32)
    out_h = nc.dram_tensor("hT_fin", [E, JB], BF16, kind="ExternalOutput")

    with tile.TileContext(nc) as tc:
        with (
            tc.tile_pool(name="const", bufs=1) as cp,
            tc.tile_pool(name="state", bufs=1) as sp,
            tc.tile_pool(name="work", bufs=2) as wp,
            tc.tile_pool(name="psum", bufs=1, space="PSUM") as pp,
        ):
            # ---- constants
            t_emb = cp.tile([E, T * BS], BF16, name="t_emb")
            nc.sync.dma_start(t_emb[:, :], din["textembT"].ap())
            t_U = cp.tile([E, E], BF16, name="t_U")
            nc.sync.dma_start(t_U[:, :], din["U"].ap())
            t_W = cp.tile([E, E], BF16, name="t_W")
            nc.sync.dma_start(t_W[:, :], din["W"].ap())
            t_I = cp.tile([E, E], BF16, name="t_I")
            nc.sync.dma_start(t_I[:, :], din["ident"].ap())
            t_ones = cp.tile([E, E], BF16, name="t_ones")
            nc.sync.dma_start(t_ones[:, :], din["ones"].ap())
            t_vm = cp.tile([E, E], BF16, name="t_vm")
            nc.sync.dma_start(t_vm[:, :], din["vmat"].ap())
            t_Wih = cp.tile([E, 3 * E], BF16, name="t_Wih")
            nc.sync.dma_start(t_Wih[:, :], din["WihT"].ap())
            t_Whh = cp.tile([E, 3 * E], BF16, name="t_Whh")
            nc.sync.dma_start(t_Whh[:, :], din["WhhT"].ap())
            t_kvub = cp.tile([E, JB], BF16, name="t_kvub")
            nc.sync.dma_start(t_kvub[:, :], din["kvub"].ap())
            t_b4 = cp.tile([unroll, (T // unroll) * JB], BF16, name="t_b4")
            nc.sync.dma_start(t_b4[:, :], din["b4"].ap())
            t_oh = cp.tile([unroll, unroll * E], BF16, name="t_oh")
            nc.sync.dma_start(t_oh[:, :], din["oh4"].ap())
            t_bc = cp.tile([E, 8], F32, name="t_bc")
            nc.sync.dma_start(t_bc[:, :], din["bcols"].ap())

            b_r = t_bc[:, 0:1]
            b_z = t_bc[:, 1:2]
            nb_z = t_bc[:, 2:3]
            bih_n = t_bc[:, 3:4]
            bhh_n = t_bc[:, 4:5]
            magic = t_bc[:, 5:6]

            # ---- state (per stream, ping-pong)
            t_h = [[sp.tile([E, W], BF16, name=f"h{s}_{i}", tag=f"h{s}_{i}")
                    for i in range(2)] for s in range(NS)]
            t_d = [[sp.tile([E, W], BF16, name=f"d{s}_{i}", tag=f"d{s}_{i}")
                    for i in range(2)] for s in range(NS)]
            for s in range(NS):
                nc.sync.dma_start(t_h[s][0][:, :],
                                  din["h0T"].ap()[:, s * W:(s + 1) * W])
                nc.sync.dma_start(t_d[s][0][:, :],
                                  din["d0T"].ap()[:, s * W:(s + 1) * W])

            # ---- PSUM banks: 4 per stream, sequential group reuse
            # pm_a: cand -> ss ; pm_b: r -> inn ; pm_c: z -> hn ; pm_g: gate
            pm_a = [pp.tile([E, W], F32, name=f"pma{s}", tag=f"pma{s}")
                    for s in range(NS)]
            pm_b = [pp.tile([E, W], F32, name=f"pmb{s}", tag=f"pmb{s}")
                    for s in range(NS)]
            pm_c = [pp.tile([E, W], F32, name=f"pmc{s}", tag=f"pmc{s}")
                    for s in range(NS)]
            pm_g = [pp.tile([E, W], F32, name=f"pmg{s}", tag=f"pmg{s}")
                    for s in range(NS)]

            def mk_shared(it, u):
                tcol = it * (unroll * BS) + u * BS
                wsl = t_emb[:, bass.ds(tcol, BS)]
                wb = wsl.unsqueeze(1).broadcast_to([E, JS, BS])
                wJT = wp.tile([E, W], BF16, tag=f"wJT{u % 2}",
                              name=f"wJT{u % 2}")
                if False:
                    nc.gpsimd.tensor_copy(
                        wJT[:, :].rearrange("e (j b) -> e j b", j=JS), wb)
                else:
                    nc.scalar.copy(
                        wJT[:, :].rearrange("e (j b) -> e j b", j=JS), wb)
                return wJT

            def step_ops(s, it, u, pi, po, wJT):
                """Return (listA, listB) of emit-closures for stream s.
                A: through omn/gate-close/d'; B: sigmoid g .. h_out."""
                hi = t_h[s][pi][:, :]
                di = t_d[s][pi][:, :]
                ho = t_h[s][po][:, :]
                do = t_d[s][po][:, :]
                ctx = {}
                A = []
                # early off-path matmuls
                A.append(lambda: nc.tensor.matmul(
                    pm_a[s][:, :], t_I[:, :], t_kvub[:, s * W:(s + 1) * W],
                    start=True, stop=False))
                A.append(lambda: nc.tensor.matmul(
                    pm_a[s][:, :], t_W[:, :], wJT[:, :],
                    start=False, stop=False))
                A.append(lambda: nc.tensor.matmul(
                    pm_g[s][:, :], t_oh[:, u * E:(u + 1) * E],
                    t_b4[:, bass.ds(it * JB + s * W, W)],
                    start=True, stop=False))
                A.append(lambda: nc.tensor.matmul(
                    pm_b[s][:, :], t_Whh[:, 0:E], di, start=True, stop=False))
                A.append(lambda: nc.tensor.matmul(
                    pm_c[s][:, :], t_Whh[:, E:2 * E], di,
                    start=True, stop=False))
                def _aw():
                    ctx["aw"] = wp.tile([E, W], BF16, tag=f"aw{s}",
                                        name=f"aw{s}")
                    eng = nc.gpsimd if False else nc.vector
                    eng.tensor_tensor(ctx["aw"][:, :], hi, wJT[:, :],
                                      op=ALU.mult)
                A.append(_aw)
                A.append(lambda: nc.tensor.matmul(
                    pm_g[s][:, :], t_ones[:, :], ctx["aw"][:, :],
                    start=False, stop=False))
                # candidate (on-path)
                A.append(lambda: nc.tensor.matmul(
                    pm_a[s][:, :], t_U[:, :], hi, start=False, stop=True))
                def _relu():
                    ctx["candi"] = wp.tile([E, W], BF16, tag=f"candi{s}",
                                           name=f"candi{s}")
                    if False:
                        nc.vector.tensor_scalar_max(ctx["candi"][:, :],
                                                    pm_a[s][:, :], 0.0)
                    else:
                        nc.scalar.activation(ctx["candi"][:, :],
                                             pm_a[s][:, :], ACT.Relu)
                A.append(_relu)
                # hn second group into pm_a (right after relu read)
                A.append(lambda: nc.tensor.matmul(
                    pm_a[s][:, :], t_Whh[:, 2 * E:], di,
                    start=True, stop=True))
                A.append(lambda: nc.tensor.matmul(
                    pm_b[s][:, :], t_Wih[:, 0:E], ctx["candi"][:, :],
                    start=False, stop=True))
                A.append(lambda: nc.tensor.matmul(
                    pm_c[s][:, :], t_Wih[:, E:2 * E], ctx["candi"][:, :],
                    start=False, stop=True))
                def _sigr():
                    ctx["rg"] = wp.tile([E, W], BF16, tag=f"rg{s}",
                                        name=f"rg{s}")
                    nc.scalar.activation(ctx["rg"][:, :], pm_b[s][:, :],
                                         ACT.Sigmoid, bias=b_r)
                A.append(_sigr)
                def _rhn():
                    if False:
                        hnb = wp.tile([E, W], BF16, tag=f"hnb{s}",
                                      name=f"hnb{s}")
                        nc.scalar.activation(hnb[:, :], pm_a[s][:, :],
                                             ACT.Identity, bias=bhh_n)
                        ctx["rhn"] = wp.tile([E, W], BF16, tag=f"rhn{s}",
                                             name=f"rhn{s}")
                        nc.vector.tensor_tensor(ctx["rhn"][:, :], hnb[:, :],
                                                ctx["rg"][:, :], op=ALU.mult)
                    else:
                        ctx["rhn"] = wp.tile([E, W], F32, tag=f"rhn{s}",
                                             name=f"rhn{s}")
                        nc.vector.scalar_tensor_tensor(
                            ctx["rhn"][:, :], pm_a[s][:, :], bhh_n,
                            ctx["rg"][:, :], op0=ALU.add, op1=ALU.mult)
                A.append(_rhn)
                def _sigz():
                    ctx["zg"] = wp.tile([E, W], BF16, tag=f"zg{s}",
                                        name=f"zg{s}")
                    nc.scalar.activation(ctx["zg"][:, :], pm_c[s][:, :],
                                         ACT.Sigmoid, bias=b_z)
                A.append(_sigz)
                def _omz():
                    ctx["omz"] = wp.tile([E, W], BF16, tag=f"omz{s}",
                                         name=f"omz{s}")
                    nc.vector.tensor_scalar(ctx["omz"][:, :], ctx["zg"][:, :],
                                            -1.0, 1.0, ALU.mult, ALU.add)
                A.append(_omz)
                # inn second group into pm_b (after sig_r read)
                A.append(lambda: nc.tensor.matmul(
                    pm_b[s][:, :], t_Wih[:, 2 * E:], ctx["candi"][:, :],
                    start=True, stop=True))
                def _tadd():
                    if False:
                        innb = wp.tile([E, W], BF16, tag=f"innb{s}",
                                       name=f"innb{s}")
                        nc.scalar.activation(innb[:, :], pm_b[s][:, :],
                                             ACT.Identity, bias=bih_n)
                        ctx["tadd"] = wp.tile([E, W], BF16, tag=f"tadd{s}",
                                              name=f"tadd{s}")
                        nc.vector.tensor_tensor(ctx["tadd"][:, :],
                                                innb[:, :],
                                                ctx["rhn"][:, :], op=ALU.add)
                    else:
                        ctx["tadd"] = wp.tile([E, W], F32, tag=f"tadd{s}",
                                              name=f"tadd{s}")
                        nc.vector.tensor_tensor(ctx["tadd"][:, :],
                                                pm_b[s][:, :],
                                                ctx["rhn"][:, :], op=ALU.add)
                A.append(_tadd)
                def _tanh():
                    ctx["ng"] = wp.tile([E, W], BF16, tag=f"ng{s}",
                                        name=f"ng{s}")
                    tb_ = (0.0 if False else bih_n)
                    nc.scalar.activation(ctx["ng"][:, :], ctx["tadd"][:, :],
                                         ACT.Tanh, bias=tb_)
                A.append(_tanh)
                def _zd():
                    ctx["zd"] = wp.tile([E, W], BF16, tag=f"zd{s}",
                                        name=f"zd{s}")
                    eng = nc.gpsimd if False else nc.vector
                    eng.tensor_tensor(ctx["zd"][:, :], ctx["zg"][:, :],
                                      di, op=ALU.mult)
                A.append(_zd)
                A.append(lambda: nc.tensor.matmul(
                    pm_g[s][:, :], t_vm[:, :], ctx["zd"][:, :],
                    start=False, stop=False))
                def _omn():
                    ctx["omn"] = wp.tile([E, W], BF16, tag=f"omn{s}",
                                         name=f"omn{s}")
                    nc.vector.tensor_tensor(ctx["omn"][:, :],
                                            ctx["omz"][:, :],
                                            ctx["ng"][:, :], op=ALU.mult)
                A.append(_omn)
                A.append(lambda: nc.tensor.matmul(
                    pm_g[s][:, :], t_vm[:, :], ctx["omn"][:, :],
                    start=False, stop=True))
                def _dblend():
                    eng = nc.gpsimd if False else nc.vector
                    eng.tensor_tensor(do, ctx["omn"][:, :], ctx["zd"][:, :],
                                      op=ALU.add)
                A.append(_dblend)

                B = []
                def _sigg():
                    ctx["gg"] = wp.tile([E, W], BF16, tag=f"gg{s}",
                                        name=f"gg{s}")
                    nc.scalar.activation(ctx["gg"][:, :], pm_g[s][:, :],
                                         ACT.Sigmoid)
                B.append(_sigg)
                def _gc():
                    ctx["gc"] = wp.tile([E, W], BF16, tag=f"gc{s}",
                                        name=f"gc{s}")
                    nc.vector.tensor_tensor(ctx["gc"][:, :], ctx["gg"][:, :],
                                            ctx["candi"][:, :], op=ALU.mult)
                B.append(_gc)
                def _h1():
                    ctx["h1"] = wp.tile([E, W], BF16, tag=f"h1{s}",
                                        name=f"h1{s}")
                    nc.vector.tensor_tensor(ctx["h1"][:, :], hi,
                                            ctx["gc"][:, :], op=ALU.add)
                B.append(_h1)
                def _sq():
                    ctx["sq"] = wp.tile([E, W], BF16, tag=f"sq{s}",
                                        name=f"sq{s}")
                    nc.vector.tensor_tensor(ctx["sq"][:, :], ctx["h1"][:, :],
                                            ctx["h1"][:, :], op=ALU.mult)
                B.append(_sq)
                # ss second group into pm_c (after sig_z/omz reads)
                B.append(lambda: nc.tensor.matmul(
                    pm_c[s][:, :], t_ones[:, :], ctx["sq"][:, :],
                    start=True, stop=True))
                def _seed():
                    ctx["y0"] = wp.tile([E, W], I32, tag=f"y0{s}",
                                        name=f"y0{s}")
                    if False:
                        sf = wp.tile([E, W], F32, tag=f"sf{s}",
                                     name=f"sf{s}")
                        nc.scalar.activation(sf[:, :],
                                             pm_c[s][:, :].bitcast(I32),
                                             ACT.Copy)
                        nc.scalar.activation(ctx["y0"][:, :], sf[:, :],
                                             ACT.Identity, bias=magic,
                                             scale=-0.5)
                    else:
                        nc.vector.scalar_tensor_tensor(
                            ctx["y0"][:, :], pm_c[s][:, :].bitcast(I32), -0.5,
                            magic.broadcast_to([E, W]),
                            op0=ALU.mult, op1=ALU.add)
                B.append(_seed)
                ynames = []
                for k in range(nr_passes):
                    def _nr(k=k):
                        yin = (ctx["y0"][:, :].bitcast(F32) if k == 0
                               else ctx[f"yn{k-1}"][:, :])
                        ysq = wp.tile([E, W], F32, tag=f"ysq{s}_{k}",
                                      name=f"ysq{s}_{k}")
                        nc.vector.tensor_tensor(ysq[:, :], yin, yin,
                                                op=ALU.mult)
                        tq = wp.tile([E, W], F32, tag=f"tq{s}_{k}",
                                     name=f"tq{s}_{k}")
                        nc.vector.scalar_tensor_tensor(
                            tq[:, :], ysq[:, :], -0.5, pm_c[s][:, :],
                            op0=ALU.mult, op1=ALU.mult)
                        ctx[f"yn{k}"] = wp.tile([E, W], F32, tag=f"yn{s}_{k}",
                                                name=f"yn{s}_{k}")
                        nc.vector.scalar_tensor_tensor(
                            ctx[f"yn{k}"][:, :], tq[:, :], 1.5, yin,
                            op0=ALU.add, op1=ALU.mult)
                    B.append(_nr)
                def _hout():
                    yfin = (ctx["y0"][:, :].bitcast(F32) if nr_passes == 0
                            else ctx[f"yn{nr_passes-1}"][:, :])
                    nc.vector.tensor_tensor(ho, ctx["h1"][:, :], yfin,
                                            op=ALU.mult)
                B.append(_hout)
                return A, B

            def emit_step(it, u, order="lock"):
                pi, po = u % 2, (u + 1) % 2
                wJT = mk_shared(it, u)
                A0, B0 = step_ops(0, it, u, pi, po, wJT)
                A1, B1 = step_ops(1, it, u, pi, po, wJT)
                if order == "lock":
                    for a, b in zip(A0 + B0, A1 + B1):
                        a(); b()
                elif order == "alt":
                    lead = u % 2
                    seqs = (A0 + B0, A1 + B1)
                    for f in seqs[lead] + seqs[1 - lead]:
                        f()
                elif order == "seq":
                    for f in A0 + B0 + A1 + B1:
                        f()
                elif order == "half":
                    for f in A0 + A1 + B0 + B1:
                        f()
                return B1 if order == "pipe" else None

            ORDER = "seq"
            with tc.For_i(0, repeat, 1) as _rep:
                with tc.For_i(0, n_steps // unroll, 1, 
                              staggered_reset=staggered) as it:
                    if ORDER == "pipe":
                        pend = None
                        for u in range(unroll):
                            pi, po = u % 2, (u + 1) % 2
                            wJT = mk_shared(it, u)
                            A0, B0 = step_ops(0, it, u, pi, po, wJT)
                            A1, B1 = step_ops(1, it, u, pi, po, wJT)
                            for f in A0:
                                f()
                            if pend is not None:
                                for f in pend:
                                    f()
                            for f in A1:
                                f()
                            for f in B0:
                                f()
                            pend = B1
                        for f in pend:
                            f()
                    else:
                        for u in range(unroll):
                            emit_step(it, u, ORDER)

            for s in range(NS):
                nc.sync.dma_start(out_h.ap()[:, s * W:(s + 1) * W],
                                  t_h[s][0][:, :])

    if split_waits:
        _split_sync_waits(nc)
    return nc


# ------------------------------------------------------------ host wrappers
_CACHE = {}


def _get_nc():
    if "nc" not in _CACHE:
        _CACHE["nc"] = _build_nc()
    return _CACHE["nc"]


def _mk_oh4():
    oh = np.zeros((UNROLL, UNROLL * E), np.float32)
    for u in range(UNROLL):
        oh[u, u * E:(u + 1) * E] = 1.0
    return oh


def _make_shared(inputs):
    f32 = np.float32
    keys = np.asarray(inputs["keys"], f32)
    Vm = np.asarray(inputs["Vm"], f32)
    v = np.asarray(inputs["v"], f32)
    bih = np.asarray(inputs["bih"], f32)
    bhh = np.asarray(inputs["bhh"], f32)
    kV = keys @ Vm                                   # [J,E]
    kvub = (np.repeat(kV.T, BS, axis=1)
            + np.asarray(inputs["U_bias"], f32)[:, None])  # [E, JB]
    bcols = np.zeros((E, 8), f32)
    bcols[:, 0] = bih[:E] + bhh[:E]
    bcols[:, 1] = bih[E:2 * E] + bhh[E:2 * E]
    bcols[:, 2] = -(bih[E:2 * E] + bhh[E:2 * E])
    bcols[:, 3] = bih[2 * E:]
    bcols[:, 4] = bhh[2 * E:]
    bcols[:, 5] = MAGICF
    consts = {
        "U": np.asarray(inputs["U"], f32).astype(BF),
        "W": np.asarray(inputs["W"], f32).astype(BF),
        "ident": np.eye(E, dtype=f32).astype(BF),
        "ones": np.ones((E, E), f32).astype(BF),
        "vmat": np.tile(v[:, None], (1, E)).astype(BF),
        "WihT": np.ascontiguousarray(np.asarray(inputs["Wih"], f32).T).astype(BF),
        "WhhT": np.ascontiguousarray(np.asarray(inputs["Whh"], f32).T).astype(BF),
        "kvub": kvub.astype(BF),
        "oh4": _mk_oh4().astype(BF),
        "bcols": bcols,
    }
    return {"consts": consts, "keys": keys,
            "h0": np.asarray(inputs["h0"], f32),
            "d0": np.asarray(inputs["d0"], f32)}


def _prep_core_inputs(c, text, emb, shared):
    bs, be = c * BS, (c + 1) * BS
    tcore = text[:, bs:be]                           # [T, BS]
    gat = emb[tcore.reshape(-1)]                     # [T*BS, E]
    textembT = np.ascontiguousarray(gat.T)           # [E, T*BS]
    # b4: gate keys.w term, b4[u, it*JB + col] = keys[j(col)] . w_{4it+u, b(col)}
    kw = shared["keys"] @ gat.T                      # [J, T*BS]
    kw = kw.reshape(J, T, BS).transpose(1, 0, 2).reshape(T, JB)  # [T, JB]
    b4 = np.ascontiguousarray(
        kw.reshape(T // UNROLL, UNROLL, JB).transpose(1, 0, 2)
        .reshape(UNROLL, (T // UNROLL) * JB))
    m = dict(shared["consts"])
    m["textembT"] = textembT.astype(BF)
    m["b4"] = b4.astype(BF)
    m["h0T"] = np.ascontiguousarray(
        shared["h0"][:, bs:be, :].transpose(2, 0, 1).reshape(E, JB)).astype(BF)
    m["d0T"] = np.ascontiguousarray(
        shared["d0"][:, bs:be, :].transpose(2, 0, 1).reshape(E, JB)).astype(BF)
    return m


def kernel(text, target, aspect, emb, keys, U, Vm, W, U_bias, v,
           Wih, Whh, bih, bhh, W_att, c1_w, c1_b, bn_g, bn_b,
           c2_w, c2_b, h0, d0):
    text = np.asarray(text)
    emb = np.asarray(emb, dtype=np.float32)
    f32 = np.float32

    shared = _make_shared(dict(
        text=text, target=target, aspect=aspect, emb=emb, keys=keys, U=U,
        Vm=Vm, W=W, U_bias=U_bias, v=v, Wih=Wih, Whh=Whh, bih=bih, bhh=bhh,
        W_att=W_att, c1_w=c1_w, c1_b=c1_b, bn_g=bn_g, bn_b=bn_b, c2_w=c2_w,
        c2_b=c2_b, h0=h0, d0=d0))

    in_maps = [_prep_core_inputs(c, text, emb, shared) for c in range(NC_)]

    nc = _get_nc()
    res = run_bass_kernel_spmd(nc, in_maps, core_ids=list(range(NC_)))
    _CACHE["last_results"] = res

    h_fin = np.zeros((J, B, E), f32)
    for c in range(NC_):
        hT = np.asarray(res.results[c]["hT_fin"]).astype(f32)   # [E, JB]
        h_fin[:, c * BS:(c + 1) * BS, :] = (
            hT.reshape(E, J, BS).transpose(1, 2, 0))

    # ---- output head (host)
    target_embed = emb[np.asarray(target)]
    aspect_embed = emb[np.asarray(aspect)]
    last_h = h_fin.transpose(1, 0, 2)               # [B,J,E]
    ta = np.concatenate([target_embed, aspect_embed], axis=1)
    att = ((np.asarray(keys) @ np.asarray(W_att)) @ ta.T).T.astype(f32)
    att = att - att.max(axis=1, keepdims=True)
    att = np.exp(att)
    att /= att.sum(axis=1, keepdims=True)
    u_read = np.einsum("bje,bj->be", last_h, att).astype(f32)
    hidden = u_read @ np.asarray(c1_w).T + c1_b + aspect_embed
    mu = hidden.mean(axis=0)
    var = hidden.var(axis=0)
    hidden = (hidden - mu) / np.sqrt(var + 1e-5) * bn_g + bn_b
    hidden = np.maximum(hidden, 0)
    return (hidden @ np.asarray(c2_w).T + c2_b).astype(f32)
